# revision 2
# baseline (speedup 1.0000x reference)
"""Causal self-attention on 8 Trainium2 NeuronCores (SPMD, no collectives).

Sharding (hardcoded): core c -> batch b = c//4, head-group g = c%4
(4 heads = 256 cols of Wq/Wk/Wv, 256 rows of Wo). Each core computes a
partial output  attended(heads of g) @ Wo[rows of g]  for its batch;
the host sums the 4 partials per batch (row-parallel unshard).

v3: the attention loop is restructured around 512-wide query columns so
the score PSUM tiles are single-bank and can be double-buffered per
parity (4 slots). HW-measured sem/chain latency made the v2 pipeline
(single-buffered 1024-wide sc) serialize PE<->ACT at ~3.9us per key
block; with double buffering the exp of block i overlaps the scores of
block i+1. Two further banks are "foreign" slots: projection and Wo
units are interleaved INTO the attention loop to fill the PE while ACT
paces the softmax. Score matmuls alternate 64-row groups 0/64 per head
parity - HW-verified to run concurrently (83ns vs 217ns per 512-col
matmul). Everything is fp16 (same PE rate as bf16, 8x the mantissa).
"""

import numpy as np
from contextlib import ExitStack

import concourse.bass as bass
import concourse.bacc as bacc
import concourse.tile as tile
from concourse import mybir
from concourse.bass_utils import run_bass_kernel_spmd

B, S, D, H, HD = 2, 2048, 1024, 16, 64
NCORES = 8
GROUPS = 4            # head-groups (cores per batch)
WCOLS = D // GROUPS   # 256 = 4 heads per core
PB = 128              # partition block
NKB = S // PB         # 16 key blocks
NDC = D // PB         # 8 contraction chunks
QC = 512              # query column width (1 PSUM bank of f32)
NQC = S // QC         # 4 query columns
f32 = mybir.dt.float32
fp16 = mybir.dt.float16
EXP = mybir.ActivationFunctionType.Exp
COPY = mybir.ActivationFunctionType.Copy

LAST_RESULTS = None

# scheduling knobs
FILLNS = 700          # foreign-work budget granted per attention i-block
WIDEP1_EVAC = "alt"   # wideP1 evacuation engine: dve | act | alt
UNIT_EVAC = "dve"     # proj/wo unit evacuation engine
AVC_ENGINE = "alt"    # attended-evac engine


def build_nc(reps=1):
    nc = bacc.Bacc("TRN2")
    xT = nc.declare_dram_parameter("xT", [D, S], fp16, isOutput=False)
    wq = nc.declare_dram_parameter("wq", [D, WCOLS], fp16, isOutput=False)
    wk = nc.declare_dram_parameter("wk", [D, WCOLS], fp16, isOutput=False)
    wv = nc.declare_dram_parameter("wv", [D, WCOLS], fp16, isOutput=False)
    wo = nc.declare_dram_parameter("wo", [WCOLS, D], fp16, isOutput=False)
    msk = nc.declare_dram_parameter("msk", [PB, PB], fp16, isOutput=False)
    idn = nc.declare_dram_parameter("idn", [PB, 64], fp16, isOutput=False)
    outp = nc.declare_dram_parameter("outp", [S, D], fp16, isOutput=True)

    with tile.TileContext(nc) as tc:
        for r in range(reps):
            with ExitStack() as ctx:
                _build(ctx, tc, nc, xT, wq, wk, wv, wo, msk, idn, outp,
                       sfx=f"_{r}" if reps > 1 else "")
    nc.compile()
    return nc


def _build(ctx, tc, nc, xT, wq, wk, wv, wo, msk, idn, outp, sfx=""):
    cst = ctx.enter_context(tc.tile_pool(name=f"cst{sfx}", bufs=1))
    qkv = ctx.enter_context(tc.tile_pool(name=f"qkv{sfx}", bufs=1))
    v1p = ctx.enter_context(tc.tile_pool(name=f"v1p{sfx}", bufs=1))
    dramp = ctx.enter_context(tc.tile_pool(name=f"dramp{sfx}", bufs=2,
                                           space="DRAM"))
    expp = ctx.enter_context(tc.tile_pool(name=f"expp{sfx}", bufs=6))
    rp = ctx.enter_context(tc.tile_pool(name=f"rp{sfx}", bufs=3))
    op = ctx.enter_context(tc.tile_pool(name=f"op{sfx}", bufs=4))
    # 8 single-bank PSUM slots: 4 score (double-buffered x 2 parities),
    # 2 attended accumulators, 2 foreign (projection/Wo/vtile units)
    psum = ctx.enter_context(tc.tile_pool(name=f"psum{sfx}", bufs=1,
                                          space="PSUM"))

    # fp16 scratch columns for the [V | 1] stationaries
    onec = cst.tile([PB, 1], fp16, tag="onec", name="onec")
    nc.vector.memset(onec, 1.0)
    zoc = cst.tile([PB, 64], fp16, tag="zoc", name="zoc")
    nc.vector.memset(zoc, 0.0)
    nc.vector.memset(zoc[:, 32:33], 1.0)
    # [one; zero] row pairs for the PE partition-broadcast normalize
    ones_bc = cst.tile([66, PB], fp16, tag="ones_bc", name="ones_bc")
    nc.vector.memset(ones_bc, 0.0)
    nc.vector.memset(ones_bc[32:33, :], 1.0)
    nc.vector.memset(ones_bc[64:65, :], 1.0)
    # ACT exp-table preload target (see dummy exp below)
    warm = cst.tile([32, 32], fp16, tag="warm", name="warm")
    nc.vector.memset(warm, 0.0)

    w_sb = {}
    xts = []

    # --- DMA order on the SP HWDGE queue: first halves of wq/wk, xt0,
    # second halves, then the rest of xT; wv/wo/msk/idn after xt1 ---
    def _w_half(name, h, half):
        if half == 0:
            t = cst.tile([PB, NDC, WCOLS], fp16, tag=name, name=name)
            w_sb[name] = t
        t = w_sb[name]
        cs = slice(0, NDC // 2) if half == 0 else slice(NDC // 2, NDC)
        src = h[half * (D // 2):(half + 1) * (D // 2), :]
        nc.sync.dma_start(out=t[:, cs, :],
                          in_=src.rearrange("(c p) m -> p c m", p=PB))

    def _load_xt(c):
        t = xtp.tile([PB, S], fp16, tag=f"xt{c}", name=f"xt{c}")
        nc.sync.dma_start(out=t, in_=xT[c * PB:(c + 1) * PB, :])
        xts.append(t)

    xtp_ctx = ExitStack()
    xtp = xtp_ctx.enter_context(tc.tile_pool(name=f"xtp{sfx}", bufs=1))

    _w_half("wq", wq, 0)
    _w_half("wk", wk, 0)
    _load_xt(0)
    _w_half("wq", wq, 1)
    _w_half("wk", wk, 1)
    _load_xt(1)
    _w_half("wv", wv, 0)
    _w_half("wv", wv, 1)
    for c in range(2, NDC):
        _load_xt(c)
    wo_sb = cst.tile([PB, 2, D], fp16, tag="wo", name="wo_sb")
    nc.sync.dma_start(out=wo_sb,
                      in_=wo[:].rearrange("(r p) n -> p r n", p=PB))
    msk_sb = cst.tile([PB, PB], fp16, tag="msk", name="msk_sb")
    nc.sync.dma_start(out=msk_sb, in_=msk[:])
    idn_sb = cst.tile([PB, 64], fp16, tag="idn", name="idn_sb")
    nc.sync.dma_start(out=idn_sb, in_=idn[:])

    qt, kt, vt = {}, {}, {}
    for m, wname in [(m, w) for m in range(2) for w in ("wq", "wk", "wv")]:
        store = {"wq": qt, "wk": kt, "wv": vt}[wname]
        store[m] = qkv.tile([PB, S], fp16, tag=f"{wname}{m}",
                            name=f"{wname}t{m}")
    stacks = {m: qkv.tile([PB, S], fp16, tag=f"stk{m}", name=f"stk{m}")
              for m in range(2)}
    v1e, v1o = {}, {}

    # preload the ACT exp spline table during the DMA-paced start
    nc.scalar.activation(warm, warm, EXP, scale=0.125)

    def _evac(dst, src, engine):
        if engine == "act":
            nc.scalar.activation(dst, src, COPY)
        else:
            nc.vector.tensor_copy(dst, src)

    SLOT8 = [f"B{j}" for j in range(8)]

    def _wide_p1():
        """(0,wq) and (0,wk) over all 4 columns, c-outer accumulation
        across all 8 banks - the PE tracks the xT DMA chunk stream."""
        pps = {}
        for j, wname in enumerate(("wq", "wk")):
            for qc in range(NQC):
                pps[(wname, qc)] = psum.tile(
                    [PB, QC], f32, tag=SLOT8[4 * j + qc], name="pp")
        for c in range(NDC):
            for wname in ("wq", "wk"):
                for qc in range(NQC):
                    nc.tensor.matmul(
                        pps[(wname, qc)],
                        w_sb[wname][:, c, 0:PB],
                        xts[c][:, qc * QC:(qc + 1) * QC],
                        start=(c == 0), stop=(c == NDC - 1),
                    )
        for j, wname in enumerate(("wq", "wk")):
            dst = {"wq": qt, "wk": kt}[wname][0]
            for qc in range(NQC):
                eng = (WIDEP1_EVAC if WIDEP1_EVAC != "alt"
                       else ("act" if qc % 2 else "dve"))
                _evac(dst[:, qc * QC:(qc + 1) * QC], pps[(wname, qc)], eng)

    # --- foreign units -------------------------------------------------
    fslot = [0]

    def _next_f():
        fslot[0] ^= 1
        return SLOT8[6 + fslot[0]]

    done_proj = set()

    def _proj_unit(m, wname, qc):
        """One [128 dims, 512 s-cols] projection output, 8 c-chunks."""
        key = (m, wname, qc)
        if key in done_proj:
            return
        done_proj.add(key)
        pp = psum.tile([PB, QC], f32, tag=_next_f(), name="pp")
        for c in range(NDC):
            nc.tensor.matmul(
                pp, w_sb[wname][:, c, m * PB:(m + 1) * PB],
                xts[c][:, qc * QC:(qc + 1) * QC],
                start=(c == 0), stop=(c == NDC - 1),
            )
        dst = {"wq": qt, "wk": kt, "wv": vt}[wname][m]
        _evac(dst[:, qc * QC:(qc + 1) * QC], pp, UNIT_EVAC)

    done_vt = set()

    def _vtile(m, i):
        if (m, i) in done_vt:
            return
        done_vt.add((m, i))
        _proj_unit(m, "wv", i // 4)
        for parity in range(2):
            off = 64 * parity
            tp = psum.tile([PB, 64], fp16, tag=_next_f(), name="tp")
            nc.tensor.transpose(
                tp,
                vt[m][off:off + 64, i * PB:(i + 1) * PB],
                idn_sb[off:off + 64, :],
            )
            if parity == 0:
                ve = v1p.tile([PB, 65], fp16, tag=f"v1e{m}_{i}",
                              name=f"v1e{m}_{i}")
                nc.vector.tensor_copy(ve[:, 0:64], tp)
                nc.vector.tensor_copy(ve[:, 64:65], onec)
                v1e[(m, i)] = ve
            else:
                vo = v1p.tile([PB, PB], fp16, tag=f"v1o{m}_{i}",
                              name=f"v1o{m}_{i}")
                nc.vector.tensor_copy(vo[:, 0:64], zoc)
                nc.vector.tensor_copy(vo[:, 64:128], tp)
                v1o[(m, i)] = vo

    def _wo_unit(sb, dh):
        """partial out block [128 s, 512 d-cols]: stack^T @ Wo rows."""
        pw = psum.tile([PB, QC], f32, tag=_next_f(), name="pw")
        for m in range(2):
            nc.tensor.matmul(
                pw,
                stacks[m][:, sb * PB:(sb + 1) * PB],
                wo_sb[:, m, dh * QC:(dh + 1) * QC],
                start=(m == 0), stop=(m == 1),
            )
        ot = op.tile([PB, QC], fp16, tag="ot", name="ot")
        _evac(ot, pw, UNIT_EVAC)
        nc.sync.dma_start(
            out=outp[sb * PB:(sb + 1) * PB, dh * QC:(dh + 1) * QC], in_=ot)

    # foreign queue: (est_ns, key, closure); keys already force-emitted
    # by dependency needs are skipped without charging the budget
    foreign = []

    def _fill(budget):
        while foreign:
            est, key, fn = foreign[0]
            if key is not None and key in done_proj:
                foreign.pop(0)
                continue
            if est > budget:
                break
            foreign.pop(0)
            fn()
            budget -= est
        return budget

    # --- attention column ----------------------------------------------
    SC = SLOT8[0:4]   # score slots: [i%2][parity]
    AE, AO = SLOT8[4], SLOT8[5]

    def _attn_col(m, qc):
        qbase = qc * QC
        qend = qbase + QC
        nkb = 4 * (qc + 1)
        # ensure inputs
        _proj_unit(m, "wq", qc)
        for t in range(qc + 1):
            _proj_unit(m, "wk", t)
        for i in range(nkb):
            _vtile(m, i)
        atts = {
            0: psum.tile([65, QC], f32, tag=AE, name="att_e"),
            1: psum.tile([PB, QC], f32, tag=AO, name="att_o"),
        }
        pend = None
        budget = 0.0

        def _flush(last_i):
            i, lo, eps = pend
            for parity in range(2):
                v1t = v1e[(m, i)] if parity == 0 else v1o[(m, i)]
                nc.tensor.matmul(
                    atts[parity][:, lo - qbase:],
                    v1t,
                    eps[parity][:, lo - qbase:],
                    start=(i == 0),
                    stop=(i == last_i),
                    skip_group_check=True,
                )

        for i in range(nkb):
            qlo = max(PB * i, qbase)
            eps = {}
            scs = {}
            # both parities' scores back-to-back: 64-row stationaries on
            # alternating row groups run concurrently on HW
            for parity in range(2):
                off = 64 * parity
                sc = psum.tile([PB, QC], f32, tag=SC[2 * (i % 2) + parity],
                               name="sc")
                nc.tensor.matmul(
                    sc[:, qlo - qbase:],
                    kt[m][off:off + 64, i * PB:(i + 1) * PB],
                    qt[m][off:off + 64, qlo:qend],
                    start=True, stop=True,
                )
                scs[parity] = sc
            for parity in range(2):
                ep = expp.tile([PB, QC], fp16, tag="ep", name="ep")
                nc.scalar.activation(
                    ep[:, qlo - qbase:], scs[parity][:, qlo - qbase:],
                    EXP, scale=0.125)
                if qlo == PB * i:     # diagonal block: causal mask
                    nc.vector.tensor_mul(
                        ep[:, qlo - qbase:qlo - qbase + PB],
                        ep[:, qlo - qbase:qlo - qbase + PB],
                        msk_sb,
                    )
                eps[parity] = ep
            if pend is not None:
                _flush(nkb - 1)
            pend = (i, qlo, eps)
            budget = _fill(budget + FILLNS)
        _flush(nkb - 1)

        # normalize into stacks: reciprocal + DMA partition-broadcast
        # (DRAM bounce on the SP queue - entirely off the attention
        # critical path; the Wo consumers run >= one column later)
        for parity in range(2):
            drow = 64 if parity == 0 else 32
            rows = slice(0, 64) if parity == 0 else slice(64, 128)
            av = rp.tile([PB, QC], fp16, tag="av", name="av")
            eng = (AVC_ENGINE if AVC_ENGINE != "alt"
                   else ("act" if parity else "dve"))
            if parity == 0:
                _evac(av[0:65, :], atts[0][0:65, :], eng)
            else:
                _evac(av[32:33, :], atts[1][32:33, :], "dve")
                _evac(av[64:128, :], atts[1][64:128, :], eng)
            rt = rp.tile([66, QC], fp16, tag="rt", name="rt")
            with nc.allow_low_precision(reason="1/denom at fp16"):
                nc.vector.reciprocal(rt[drow:drow + 1, :],
                                     av[drow:drow + 1, :])
            dr = dramp.tile([1, QC], fp16, tag="dr", name="dr")
            nc.sync.dma_start(out=dr, in_=rt[drow:drow + 1, :])
            rb = rp.tile([PB, QC], fp16, tag="rb", name="rb")
            bsrc = bass.AP(
                tensor=dr.tensor, offset=dr.offset,
                ap=[[0, 64]] + [list(d) for d in dr.ap[1:]],
            )
            nc.sync.dma_start(out=rb[rows, :], in_=bsrc)
            nc.vector.tensor_mul(
                stacks[m][rows, qbase:qend], av[rows, :], rb[rows, :]
            )

    # --- emission ------------------------------------------------------
    _wide_p1()
    # foreign queue seeded with the remaining m=0/m=1 projections;
    # wo units are appended as their stacks become available
    for qc2 in range(NQC):
        for w2 in ("wq", "wk", "wv"):
            foreign.append((1800, (1, w2, qc2),
                            (lambda m=1, w=w2, q=qc2:
                             _proj_unit(m, w, q))))
    for qc in range(NQC):
        for m in range(2):
            _attn_col(m, qc)
        for sb in range(4 * qc, 4 * qc + 4):
            for dh in range(2):
                foreign.append((600, None,
                                (lambda s=sb, d=dh: _wo_unit(s, d))))
    # drain the tail
    while foreign:
        est, key, fn = foreign.pop(0)
        if key is not None and key in done_proj:
            continue
        fn()
    xtp_ctx.close()


def make_in_maps(x, Wq, Wk, Wv, Wo):
    x = np.asarray(x, dtype=np.float32)
    Wq = np.asarray(Wq, dtype=np.float16)
    Wk = np.asarray(Wk, dtype=np.float16)
    Wv = np.asarray(Wv, dtype=np.float16)
    Wo = np.asarray(Wo, dtype=np.float16)
    msk = np.triu(np.ones((PB, PB), dtype=np.float16))
    idn = np.concatenate([np.eye(64)] * 2, axis=0).astype(np.float16)
    in_maps = []
    for c in range(NCORES):
        b, g = divmod(c, GROUPS)
        in_maps.append({
            "xT": np.ascontiguousarray(x[b].T.astype(np.float16)),
            "wq": np.ascontiguousarray(Wq[:, g * WCOLS:(g + 1) * WCOLS]),
            "wk": np.ascontiguousarray(Wk[:, g * WCOLS:(g + 1) * WCOLS]),
            "wv": np.ascontiguousarray(Wv[:, g * WCOLS:(g + 1) * WCOLS]),
            "wo": np.ascontiguousarray(Wo[g * WCOLS:(g + 1) * WCOLS, :]),
            "msk": msk,
            "idn": idn,
        })
    return in_maps


def _combine(outs):
    outs = [np.asarray(o).astype(np.float32) for o in outs]
    out = np.empty((B, S, D), dtype=np.float32)
    out[0] = outs[0] + outs[1] + outs[2] + outs[3]
    out[1] = outs[4] + outs[5] + outs[6] + outs[7]
    return out


def kernel(x, Wq, Wk, Wv, Wo):
    global LAST_RESULTS
    nc = build_nc()
    in_maps = make_in_maps(x, Wq, Wk, Wv, Wo)
    res = run_bass_kernel_spmd(nc, in_maps, list(range(NCORES)))
    LAST_RESULTS = res
    return _combine([r["outp"] for r in res.results])


def _make_runner(nc, in_maps):
    """Set up a device-resident one-dispatch runner for a prebuilt nc.

    Returns (run, fetch): run() executes one dispatch and returns wall
    seconds; fetch() returns the combined full-shape output of the last
    run."""
    import time
    import jax
    from jax.sharding import Mesh, NamedSharding, PartitionSpec
    from jax.experimental.shard_map import shard_map
    from concourse import bass2jax

    bass2jax.install_neuronx_cc_hook()

    partition_name = (
        nc.partition_id_tensor.name if nc.partition_id_tensor else None
    )
    in_names, out_names, out_avals, zero_outs = [], [], [], []
    for alloc in nc.m.functions[0].allocations:
        if not isinstance(alloc, mybir.MemoryLocationSet):
            continue
        name = alloc.memorylocations[0].name
        if alloc.kind == "ExternalInput":
            if name != partition_name:
                in_names.append(name)
        elif alloc.kind == "ExternalOutput":
            out_names.append(name)
            shape = tuple(alloc.tensor_shape)
            dtype = mybir.dt.np(alloc.dtype)
            out_avals.append(jax.core.ShapedArray(shape, dtype))
            zero_outs.append(np.zeros(shape, dtype))
    n_params = len(in_names)
    n_outs = len(out_names)
    all_names = in_names + out_names
    if partition_name is not None:
        all_names = all_names + [partition_name]

    def _body(*args):
        operands = list(args)
        if partition_name is not None:
            operands.append(bass2jax.partition_id_tensor())
        return tuple(bass2jax._bass_exec_p.bind(
            *operands,
            out_avals=tuple(out_avals),
            in_names=tuple(all_names),
            out_names=tuple(out_names),
            lowering_input_output_aliases=(),
            sim_require_finite=True,
            sim_require_nnan=True,
            nc=nc,
        ))

    devices = jax.devices()[:NCORES]
    mesh = Mesh(np.asarray(devices), ("core",))
    sharded = jax.jit(
        shard_map(_body, mesh=mesh,
                  in_specs=(PartitionSpec("core"),) * (n_params + n_outs),
                  out_specs=(PartitionSpec("core"),) * n_outs,
                  check_rep=False),
        donate_argnums=tuple(range(n_params, n_params + n_outs)),
        keep_unused=True,
    )
    sh = NamedSharding(mesh, PartitionSpec("core"))
    dev_in = [
        jax.device_put(
            np.concatenate(
                [np.asarray(in_maps[c][nm]) for c in range(NCORES)], axis=0),
            sh)
        for nm in in_names
    ]
    state = {"outs": None}

    def run():
        dev_zeros = [
            jax.device_put(
                np.zeros((NCORES * z.shape[0], *z.shape[1:]), z.dtype), sh)
            for z in zero_outs
        ]
        jax.block_until_ready(dev_zeros)
        jax.block_until_ready(dev_in)
        t0 = time.perf_counter()
        outs = sharded(*dev_in, *dev_zeros)
        jax.block_until_ready(outs)
        state["outs"] = outs
        return time.perf_counter() - t0

    def fetch():
        i = out_names.index("outp")
        arr = np.asarray(state["outs"][i]).reshape(NCORES, S, D)
        return _combine([arr[c] for c in range(NCORES)])

    return run, fetch




def bench_hw_time(x, Wq, Wk, Wv, Wo, reps=65, iters=24):
    nc1 = build_nc(reps=1)
    ncR = build_nc(reps=reps)
    in_maps = make_in_maps(x, Wq, Wk, Wv, Wo)
    run1, fetch1 = _make_runner(nc1, in_maps)
    runR, fetchR = _make_runner(ncR, in_maps)
    t1, tR = [], []
    for _ in range(iters):
        t1.append(run1())
        tR.append(runR())
    out1, outR = fetch1(), fetchR()
    m1, mR = min(t1), min(tR)
    hw_time = (mR - m1) / (reps - 1)
    diag = {"t1": t1, "tR": tR, "min1": m1, "minR": mR, "reps": reps,
            "outR": outR}
    return hw_time, out1, diag
